# revision 12
# baseline (speedup 1.0000x reference)
"""Causal self-attention (B=2, L=2048, C=1024, 16 heads) on 8 Trainium2
NeuronCores via Bass/Tile.

Sharding (8 cores = 2 batches x 4 head-groups of 4 heads):
  core c: batch b = c // 4, head-group g = c % 4  (heads 4g..4g+3)

Host prep per core: x[b].T, column slices of Wq/Wk/Wv (transposed), an
output-channel slice of Wp (transposed), an additive key-mask derived from
attn_mask, and a 128x128 upper-triangular causal mask tile.  Matmul operands
are cast to bf16 on the host; accumulation is fp32 on-device (measured
output relative error vs the fp32 reference: ~4.3e-3; an fp32r build was
validated at 2.7e-4 but runs the PE at 1/4 rate).

Device program (SPMD — identical program on all cores, per-core data):
  - v = x @ Wv_slice.T in natural [token, dim] layout with a ones column per
    head appended, so the P@V matmul also produces the softmax denominator.
  - q^T,k^T [256, L]: two tiles, 2 heads each at partition offsets 0/64, via
    matmuls contracting C on the partition axis (x arrives pre-transposed).
  - attention per head-pair, per 512-query chunk, per 128-key block:
    s^T = k^T.T @ q^T (keys on partitions, queries free) -> exp on ScalarE
    with scale=1/8 and per-key-partition bias=attn_mask fused into the
    activation (scores are O(3), so no max-subtraction is needed) -> P^T in
    bf16; causal masking = memset of fully-masked query columns plus one
    128x128 triangular multiply on the diagonal block; the two heads of a
    pair use PE row groups 0-63/64-127 so their K=64 score matmuls run
    concurrently.  y^T_aug [65, chunk] = v_aug.T @ P^T accumulates over key
    blocks in PSUM; rows 0-63 are divided by row 64 (the denominator) via
    reciprocal + partition-broadcast.
  - per query chunk: 4-rank AllGather of y^T columns within the batch group
    (overlaps the next chunk's attention) -> y_full^T (1024, chunk);
    projection out[chunk, 256g:256(g+1)] = y_full @ Wp.T slice
    (output-channel-sharded so the program stays rank-uniform).
Host assembly: concatenate the 4 column slices per batch.
"""
import sys
import numpy as np
import ml_dtypes

for _p in ("/opt/trn_rl_repo",):
    if _p not in sys.path:
        sys.path.insert(0, _p)

import concourse.bass as bass
import concourse.mybir as mybir
import concourse.tile as tile
from concourse import bacc
from concourse import bass_utils

F32 = mybir.dt.float32
BF16 = mybir.dt.bfloat16
AF = mybir.ActivationFunctionType

N_CORES = 8
B, L, C, H, D = 2, 2048, 1024, 16, 64
H_PER_CORE = 4
DQ = H_PER_CORE * D          # 256 = per-core q/k/v width and out-column slice
CT = C // 128                # contraction tiles
TT = L // 128                # token tiles
QCHUNK = 512
QC = L // QCHUNK
NS = QCHUNK // 512           # 512-wide sub-chunks per query chunk
NB = L // 128                # key blocks
NEG = -30000.0


def build_kernel(use_collective=True, reps=1, phase="full"):
    nc = bacc.Bacc("TRN2", target_bir_lowering=False, debug=False,
                   num_devices=N_CORES)

    xT_d = nc.dram_tensor("xT", [C, L], BF16, kind="ExternalInput")
    wqT_d = nc.dram_tensor("wqT", [C, DQ], BF16, kind="ExternalInput")
    wkT_d = nc.dram_tensor("wkT", [C, DQ], BF16, kind="ExternalInput")
    wvT_d = nc.dram_tensor("wvT", [C, DQ], BF16, kind="ExternalInput")
    wpT_d = nc.dram_tensor("wpT", [C, DQ], BF16, kind="ExternalInput")
    kmask_d = nc.dram_tensor("kmask", [128, NB], F32, kind="ExternalInput")
    ident_d = nc.dram_tensor("ident", [128, 128], BF16, kind="ExternalInput")
    maskadd_d = nc.dram_tensor("maskadd", [128, 128], BF16, kind="ExternalInput")
    out_d = nc.dram_tensor("out", [L, DQ], F32, kind="ExternalOutput")

    with tile.TileContext(nc) as tc:
        import contextlib
        with contextlib.ExitStack() as ctx:
            const = ctx.enter_context(tc.tile_pool(name="const", bufs=1))
            kmask = const.tile([128, NB], F32)
            ident = const.tile([128, 128], BF16)
            maskadd = const.tile([128, 128], BF16)
            nc.sync.dma_start(out=kmask[:], in_=kmask_d[:])
            nc.sync.dma_start(out=ident[:], in_=ident_d[:])
            nc.sync.dma_start(out=maskadd[:], in_=maskadd_d[:])

            w_pool = ctx.enter_context(tc.tile_pool(name="w", bufs=1))
            sb = ctx.enter_context(tc.tile_pool(name="sb", bufs=1))
            pt_pool = ctx.enter_context(tc.tile_pool(name="pt", bufs=6))
            den_pool = ctx.enter_context(tc.tile_pool(name="den", bufs=3))
            stg = ctx.enter_context(tc.tile_pool(name="stg", bufs=3))
            ps = ctx.enter_context(tc.tile_pool(name="ps", bufs=1, space="PSUM"))

            wq = [w_pool.tile([128, DQ], BF16, tag=f"wq{k}", name=f"wq{k}")
                  for k in range(CT)]
            wk = [w_pool.tile([128, DQ], BF16, tag=f"wk{k}", name=f"wk{k}")
                  for k in range(CT)]
            wv = [w_pool.tile([128, DQ], BF16, tag=f"wv{k}", name=f"wv{k}")
                  for k in range(CT)]
            wp = [w_pool.tile([128, DQ], BF16, tag=f"wp{k}", name=f"wp{k}")
                  for k in range(CT)]
            for k in range(CT):
                nc.sync.dma_start(out=wq[k][:], in_=wqT_d[k*128:(k+1)*128, :])
                nc.sync.dma_start(out=wk[k][:], in_=wkT_d[k*128:(k+1)*128, :])
                nc.sync.dma_start(out=wv[k][:], in_=wvT_d[k*128:(k+1)*128, :])
                nc.sync.dma_start(out=wp[k][:], in_=wpT_d[k*128:(k+1)*128, :])

            qT = [sb.tile([128, L], BF16, tag=f"qT{m}", name=f"qT{m}")
                  for m in range(2)]
            kT = [sb.tile([128, L], BF16, tag=f"kT{m}", name=f"kT{m}")
                  for m in range(2)]
            vaug = [sb.tile([128, H_PER_CORE, D + 1], BF16, tag=f"va{t}",
                            name=f"va{t}") for t in range(TT)]
            yT = [sb.tile([128, L], BF16, tag=f"yT{m}", name=f"yT{m}")
                  for m in range(2)]
            xT = [sb.tile([128, L], BF16, tag=f"xT{k}", name=f"xTs{k}")
                  for k in range(CT)]
            yF = [sb.tile([128, L], BF16, tag=f"yF{k}", name=f"yF{k}")
                  for k in range(CT)]

            dram = ctx.enter_context(tc.tile_pool(name="dram", bufs=1,
                                                  space="DRAM"))
            ag_ins = [dram.tile([2 * 128, QCHUNK], BF16, tag=f"agi{q}",
                                name=f"agi{q}") for q in range(QC)]
            ag_outs = [dram.tile([8 * 128, QCHUNK], BF16, tag=f"ago{q}",
                                 name=f"ago{q}") for q in range(QC)]

            def qk_proj(ht):
                for t4 in range(L // 512):
                    sl = bass.ts(t4, 512)
                    for dst, w in ((qT, wq), (kT, wk)):
                        p = ps.tile([128, 512], F32, tag="psBV", name="psB",
                                    bufs=2)
                        for k in range(CT):
                            nc.tensor.matmul(
                                p[:], w[k][:, ht*128:(ht+1)*128], xT[k][:, sl],
                                start=(k == 0), stop=(k == CT - 1))
                        nc.vector.tensor_copy(dst[ht][:, sl], p[:])

            def v_proj():
                for t in range(TT):
                    nc.vector.memset(vaug[t][:, :, D:D+1], 1.0)
                    p = ps.tile([128, DQ], F32, tag="psBV", name="psV", bufs=2)
                    for k in range(CT):
                        nc.tensor.matmul(
                            p[:], xT[k][:, t*128:(t+1)*128], wv[k][:],
                            start=(k == 0), stop=(k == CT - 1))
                    nc.vector.tensor_copy(
                        vaug[t][:, :, 0:D],
                        p.rearrange("p (h d) -> p h d", h=H_PER_CORE))

            def attention(ht, qc):
                q0 = qc * QCHUNK
                nkb = (q0 + QCHUNK) // 128
                psys = {}
                for hp in (0, 64):
                    psys[hp] = ps.tile([128, QCHUNK], F32, tag=f"psy{hp}",
                                       name=f"psy{hp}", bufs=1)
                # software-pipelined: score matmuls for key block j+1 are
                # emitted before the P@V matmuls of block j, so the strict-
                # FIFO PE has independent work while ScalarE runs exp(j).
                # Score/exp/P@V all operate on the query sub-range
                # [c_lo:QCHUNK] that block j can actually attend to; the
                # causal mask of the diagonal 128-block is accumulated into
                # PSUM by an identity-matmul of a -2.4e5 triangular tile.
                def emit_s(j):
                    c_lo = max(0, j * 128 - q0)
                    pss = ps.tile([128, 2, QCHUNK], F32, tag="pssP",
                                  name="pssP", bufs=2)
                    diag = j * 128 >= q0
                    for hp in (0, 64):
                        nc.tensor.matmul(
                            pss[:, hp // 64, c_lo:QCHUNK],
                            kT[ht][hp:hp+64, j*128:(j+1)*128],
                            qT[ht][hp:hp+64, bass.ds(q0 + c_lo, QCHUNK - c_lo)],
                            start=True, stop=not diag)
                    if diag:
                        for hpi in (0, 1):
                            nc.tensor.matmul(
                                pss[:, hpi, c_lo:c_lo+128],
                                ident[:], maskadd[:],
                                start=False, stop=True)
                    return pss

                pss_j = emit_s(0)
                for j in range(nkb):
                    c_lo = max(0, j * 128 - q0)
                    pss, pss_j = pss_j, None
                    pt = pt_pool.tile([128, 2, QCHUNK], BF16,
                                      tag="ptP", name="ptP")
                    nc.scalar.activation(
                        pt[:, :, c_lo:QCHUNK], pss[:, :, c_lo:QCHUNK],
                        AF.Exp, bias=kmask[:, j:j+1], scale=0.125)
                    if j + 1 < nkb:
                        pss_j = emit_s(j + 1)
                    for hp in (0, 64):
                        h = 2 * ht + hp // 64
                        nc.tensor.matmul(
                            psys[hp][:65, c_lo:QCHUNK],
                            vaug[j][:, h, :],
                            pt[:, hp // 64, c_lo:QCHUNK],
                            start=(j == 0), stop=(j == nkb - 1))
                for hp in (0, 64):
                    psy = psys[hp]
                    qsl = bass.ds(q0, QCHUNK)
                    rden = den_pool.tile([1, QCHUNK], F32, tag="rden")
                    nc.vector.reciprocal(rden[:], psy[64:65, :])
                    rdb = den_pool.tile([64, QCHUNK], F32, tag="rdb")
                    nc.gpsimd.partition_broadcast(rdb[:], rden[:])
                    nc.vector.tensor_mul(yT[ht][hp:hp+64, qsl],
                                         psy[0:64, :], rdb[:])

            def ag_part(qc):
                q0 = qc * QCHUNK
                csl = bass.ds(q0, QCHUNK)
                ag_in = ag_ins[qc]
                for m in range(2):
                    nc.sync.dma_start(out=ag_in[m*128:(m+1)*128, :],
                                      in_=yT[m][:, csl])
                if use_collective:
                    nc.gpsimd.collective_compute(
                        "AllGather", mybir.AluOpType.bypass,
                        ins=[ag_in[:]], outs=[ag_outs[qc][:]],
                        replica_groups=[[0, 1, 2, 3], [4, 5, 6, 7]])

            def proj_part(qc):
                q0 = qc * QCHUNK
                csl = bass.ds(q0, QCHUNK)
                for k in range(CT):
                    src = ag_outs[qc] if use_collective else ag_ins[qc]
                    ksrc = k if use_collective else (k % 2)
                    nc.sync.dma_start(out=yF[k][:, csl],
                                      in_=src[ksrc*128:(ksrc+1)*128, :])
                for m in range(q0 // 128, (q0 + QCHUNK) // 128):
                    p = ps.tile([128, DQ], F32, tag="psBV", name="psP", bufs=2)
                    for k in range(CT):
                        nc.tensor.matmul(
                            p[:], yF[k][:, m*128:(m+1)*128], wp[k][:],
                            start=(k == 0), stop=(k == CT - 1))
                    st = stg.tile([128, DQ], F32, tag="st")
                    nc.vector.tensor_copy(st[:], p[:])
                    nc.sync.dma_start(out=out_d[m*128:(m+1)*128, :], in_=st[:])

            for _rep in range(reps):
                for k in range(CT):
                    nc.sync.dma_start(out=xT[k][:], in_=xT_d[k*128:(k+1)*128, :])
                v_proj()
                qk_proj(0)
                qk_proj(1)
                if phase == "proj":
                    continue
                # out-projection of chunk qc-1 is emitted between the two
                # attention halves of chunk qc, so the AllGather's DRAM
                # round-trip is covered by attention compute instead of
                # stalling the in-order PE queue.
                for qc in range(QC):
                    attention(0, qc)
                    if phase != "attn" and qc > 0:
                        proj_part(qc - 1)
                    attention(1, qc)
                    if phase != "attn":
                        ag_part(qc)
                if phase != "attn":
                    proj_part(QC - 1)

    nc.compile()
    return nc


def host_inputs(x, attn_mask, Wq, Wk, Wv, Wp):
    x = np.asarray(x)
    attn_mask = np.asarray(attn_mask)
    Wq, Wk, Wv, Wp = (np.asarray(a) for a in (Wq, Wk, Wv, Wp))

    def bfc(a):
        return np.ascontiguousarray(
            np.asarray(a, dtype=np.float32)).astype(ml_dtypes.bfloat16)

    ident = np.eye(128, dtype=np.float32).astype(ml_dtypes.bfloat16)
    r = np.arange(128)
    madd = np.where(r[:, None] > r[None, :], 8.0 * NEG, 0.0).astype(np.float32)
    madd = madd.astype(ml_dtypes.bfloat16)
    in_maps = []
    for c in range(N_CORES):
        b, g = c // 4, c % 4
        sl = slice(DQ * g, DQ * (g + 1))
        km = np.where(attn_mask[b] != 0, 0.0, NEG).astype(np.float32)
        km = np.ascontiguousarray(km.reshape(NB, 128).T)
        in_maps.append({
            "xT": bfc(x[b].T),
            "wqT": bfc(Wq[sl, :].T),
            "wkT": bfc(Wk[sl, :].T),
            "wvT": bfc(Wv[sl, :].T),
            "wpT": bfc(Wp[sl, :].T),
            "kmask": km,
            "ident": np.ascontiguousarray(ident),
            "maskadd": np.ascontiguousarray(madd),
        })
    return in_maps


_CACHED = {}


def kernel(x, attn_mask, Wq, Wk, Wv, Wp):
    if "nc" not in _CACHED:
        _CACHED["nc"] = build_kernel()
    nc = _CACHED["nc"]
    in_maps = host_inputs(x, attn_mask, Wq, Wk, Wv, Wp)
    res = bass_utils.run_bass_kernel_spmd(
        nc, in_maps, core_ids=list(range(N_CORES)))
    out = np.zeros((B, L, C), np.float32)
    for b in range(B):
        out[b] = np.concatenate(
            [res.results[4*b + g]["out"] for g in range(4)], axis=1)
    return out



# revision 17
# speedup vs baseline: 1.0859x; 1.0859x over previous
"""Causal self-attention (B=2, L=2048, C=1024, 16 heads) on 8 Trainium2
NeuronCores via Bass/Tile.

Sharding (8 cores = 2 batches x 4 head-groups of 4 heads):
  core c: batch b = c // 4, head-group g = c % 4  (heads 4g..4g+3)

Host prep per core: x[b].T, column slices of Wq/Wk/Wv (transposed), an
output-channel slice of Wp (transposed), an additive key-mask derived from
attn_mask, and a 128x128 upper-triangular causal mask tile.  Matmul operands
are cast to bf16 on the host; accumulation is fp32 on-device (measured
output relative error vs the fp32 reference: ~4.3e-3; an fp32r build was
validated at 2.7e-4 but runs the PE at 1/4 rate).

Device program (SPMD — identical program on all cores, per-core data):
  - v = x @ Wv_slice.T in natural [token, dim] layout with a ones column per
    head appended, so the P@V matmul also produces the softmax denominator.
  - q^T,k^T [256, L]: two tiles, 2 heads each at partition offsets 0/64, via
    matmuls contracting C on the partition axis (x arrives pre-transposed).
  - attention per head-pair, per 512-query chunk, per 128-key block:
    s^T = k^T.T @ q^T (keys on partitions, queries free) -> exp on ScalarE
    with scale=1/8 and per-key-partition bias=attn_mask fused into the
    activation (scores are O(3), so no max-subtraction is needed) -> P^T in
    bf16; causal masking = memset of fully-masked query columns plus one
    128x128 triangular multiply on the diagonal block; the two heads of a
    pair use PE row groups 0-63/64-127 so their K=64 score matmuls run
    concurrently.  y^T_aug [65, chunk] = v_aug.T @ P^T accumulates over key
    blocks in PSUM; rows 0-63 are divided by row 64 (the denominator) via
    reciprocal + partition-broadcast.
  - per query chunk: 4-rank AllGather of y^T columns within the batch group
    (overlaps the next chunk's attention) -> y_full^T (1024, chunk);
    projection out[chunk, 256g:256(g+1)] = y_full @ Wp.T slice
    (output-channel-sharded so the program stays rank-uniform).
Host assembly: concatenate the 4 column slices per batch.
"""
import sys
import numpy as np
import ml_dtypes

for _p in ("/opt/trn_rl_repo",):
    if _p not in sys.path:
        sys.path.insert(0, _p)

import concourse.bass as bass
import concourse.mybir as mybir
import concourse.tile as tile
from concourse import bacc
from concourse import bass_utils

F32 = mybir.dt.float32
BF16 = mybir.dt.bfloat16
AF = mybir.ActivationFunctionType

N_CORES = 8
B, L, C, H, D = 2, 2048, 1024, 16, 64
H_PER_CORE = 4
DQ = H_PER_CORE * D          # 256 = per-core q/k/v width and out-column slice
CT = C // 128                # contraction tiles
TT = L // 128                # token tiles
QCHUNK = 512
QC = L // QCHUNK
NS = QCHUNK // 512           # 512-wide sub-chunks per query chunk
NB = L // 128                # key blocks
NEG = -30000.0


def build_kernel(use_collective=True, reps=1, phase="full"):
    nc = bacc.Bacc("TRN2", target_bir_lowering=False, debug=False,
                   num_devices=N_CORES)

    xT_d = nc.dram_tensor("xT", [C, L], BF16, kind="ExternalInput")
    wqT_d = nc.dram_tensor("wqT", [C, DQ], BF16, kind="ExternalInput")
    wkT_d = nc.dram_tensor("wkT", [C, DQ], BF16, kind="ExternalInput")
    wvT_d = nc.dram_tensor("wvT", [C, DQ], BF16, kind="ExternalInput")
    wpT_d = nc.dram_tensor("wpT", [C, DQ], BF16, kind="ExternalInput")
    kmask_d = nc.dram_tensor("kmask", [128, NB], F32, kind="ExternalInput")
    ident_d = nc.dram_tensor("ident", [128, 128], BF16, kind="ExternalInput")
    maskadd_d = nc.dram_tensor("maskadd", [128, 128], BF16, kind="ExternalInput")
    out_d = nc.dram_tensor("out", [L, DQ], F32, kind="ExternalOutput")

    with tile.TileContext(nc) as tc:
        import contextlib
        with contextlib.ExitStack() as ctx:
            const = ctx.enter_context(tc.tile_pool(name="const", bufs=1))
            kmask = const.tile([128, NB], F32)
            ident = const.tile([128, 128], BF16)
            maskadd = const.tile([128, 128], BF16)
            nc.sync.dma_start(out=kmask[:], in_=kmask_d[:])
            nc.sync.dma_start(out=ident[:], in_=ident_d[:])
            nc.sync.dma_start(out=maskadd[:], in_=maskadd_d[:])

            w_pool = ctx.enter_context(tc.tile_pool(name="w", bufs=1))
            sb = ctx.enter_context(tc.tile_pool(name="sb", bufs=1))
            pt_pool = ctx.enter_context(tc.tile_pool(name="pt", bufs=6))
            den_pool = ctx.enter_context(tc.tile_pool(name="den", bufs=3))
            stg = ctx.enter_context(tc.tile_pool(name="stg", bufs=3))
            ps = ctx.enter_context(tc.tile_pool(name="ps", bufs=1, space="PSUM"))

            # weights as single [128, CT, DQ] tiles -> one DMA each
            wq = w_pool.tile([128, CT, DQ], BF16, tag="wq", name="wq")
            wk = w_pool.tile([128, CT, DQ], BF16, tag="wk", name="wk")
            wv = w_pool.tile([128, CT, DQ], BF16, tag="wv", name="wv")
            wp = w_pool.tile([128, CT, DQ], BF16, tag="wp", name="wp")
            for w, wd in ((wq, wqT_d), (wk, wkT_d), (wv, wvT_d), (wp, wpT_d)):
                nc.sync.dma_start(
                    out=w[:], in_=wd[:].rearrange("(k p) d -> p k d", p=128))

            qT = [sb.tile([128, L], BF16, tag=f"qT{m}", name=f"qT{m}")
                  for m in range(2)]
            kT = [sb.tile([128, L], BF16, tag=f"kT{m}", name=f"kT{m}")
                  for m in range(2)]
            vaug = [sb.tile([128, H_PER_CORE, D + 1], BF16, tag=f"va{t}",
                            name=f"va{t}") for t in range(TT)]
            yT = sb.tile([128, 2, L], BF16, tag="yT", name="yT")
            xT = sb.tile([128, CT, L], BF16, tag="xT", name="xTs")
            yF = sb.tile([128, CT, L], BF16, tag="yF", name="yF")

            dram = ctx.enter_context(tc.tile_pool(name="dram", bufs=1,
                                                  space="DRAM"))
            ag_ins = [dram.tile([2 * 128, QCHUNK], BF16, tag=f"agi{q}",
                                name=f"agi{q}") for q in range(QC)]
            ag_outs = [dram.tile([8 * 128, QCHUNK], BF16, tag=f"ago{q}",
                                 name=f"ago{q}") for q in range(QC)]

            def qk_proj(ht):
                for t4 in range(L // 512):
                    sl = bass.ts(t4, 512)
                    for dst, w in ((qT, wq), (kT, wk)):
                        p = ps.tile([128, 512], F32, tag="psBV", name="psB",
                                    bufs=2)
                        for k in range(CT):
                            nc.tensor.matmul(
                                p[:], w[:, k, ht*128:(ht+1)*128],
                                xT[:, k, sl],
                                start=(k == 0), stop=(k == CT - 1))
                        nc.vector.tensor_copy(dst[ht][:, sl], p[:])

            def v_proj():
                for t in range(TT):
                    nc.vector.memset(vaug[t][:, :, D:D+1], 1.0)
                    p = ps.tile([128, DQ], F32, tag="psBV", name="psV", bufs=2)
                    for k in range(CT):
                        nc.tensor.matmul(
                            p[:], xT[:, k, t*128:(t+1)*128], wv[:, k, :],
                            start=(k == 0), stop=(k == CT - 1))
                    nc.vector.tensor_copy(
                        vaug[t][:, :, 0:D],
                        p.rearrange("p (h d) -> p h d", h=H_PER_CORE))

            def attention(ht, qc):
                q0 = qc * QCHUNK
                nkb = (q0 + QCHUNK) // 128
                psys = {}
                for hp in (0, 64):
                    psys[hp] = ps.tile([128, QCHUNK], F32, tag=f"psy{hp}",
                                       name=f"psy{hp}", bufs=1)
                # software-pipelined: score matmuls for key block j+1 are
                # emitted before the P@V matmuls of block j, so the strict-
                # FIFO PE has independent work while ScalarE runs exp(j).
                # Score/exp/P@V all operate on the query sub-range
                # [c_lo:QCHUNK] that block j can actually attend to; the
                # causal mask of the diagonal 128-block is accumulated into
                # PSUM by an identity-matmul of a -2.4e5 triangular tile.
                def emit_s(j):
                    c_lo = max(0, j * 128 - q0)
                    pss = ps.tile([128, 2, QCHUNK], F32, tag="pssP",
                                  name="pssP", bufs=2)
                    diag = j * 128 >= q0
                    for hp in (0, 64):
                        nc.tensor.matmul(
                            pss[:, hp // 64, c_lo:QCHUNK],
                            kT[ht][hp:hp+64, j*128:(j+1)*128],
                            qT[ht][hp:hp+64, bass.ds(q0 + c_lo, QCHUNK - c_lo)],
                            start=True, stop=not diag)
                    if diag:
                        for hpi in (0, 1):
                            nc.tensor.matmul(
                                pss[:, hpi, c_lo:c_lo+128],
                                ident[:], maskadd[:],
                                start=False, stop=True)
                    return pss

                pss_j = emit_s(0)
                for j in range(nkb):
                    c_lo = max(0, j * 128 - q0)
                    pss, pss_j = pss_j, None
                    pt = pt_pool.tile([128, 2, QCHUNK], BF16,
                                      tag="ptP", name="ptP")
                    nc.scalar.activation(
                        pt[:, :, c_lo:QCHUNK], pss[:, :, c_lo:QCHUNK],
                        AF.Exp, bias=kmask[:, j:j+1], scale=0.125)
                    if j + 1 < nkb:
                        pss_j = emit_s(j + 1)
                    for hp in (0, 64):
                        h = 2 * ht + hp // 64
                        nc.tensor.matmul(
                            psys[hp][:65, c_lo:QCHUNK],
                            vaug[j][:, h, :],
                            pt[:, hp // 64, c_lo:QCHUNK],
                            start=(j == 0), stop=(j == nkb - 1))
                for hp in (0, 64):
                    psy = psys[hp]
                    qsl = bass.ds(q0, QCHUNK)
                    # evict PSUM via a copy so the (single-buffered) psy
                    # tile is free for the next chunk while the den chain
                    # (recip -> partition-broadcast -> mul) runs on the copy
                    yraw = den_pool.tile([65, QCHUNK], F32, tag="yraw")
                    nc.vector.tensor_copy(yraw[:], psy[:65, :])
                    rden = den_pool.tile([1, QCHUNK], F32, tag="rden")
                    nc.vector.reciprocal(rden[:], yraw[64:65, :])
                    rdb = den_pool.tile([64, QCHUNK], F32, tag="rdb")
                    nc.gpsimd.partition_broadcast(rdb[:], rden[:])
                    nc.vector.tensor_mul(yT[hp:hp+64, ht, qsl],
                                         yraw[0:64, :], rdb[:])

            def ag_part(qc):
                q0 = qc * QCHUNK
                csl = bass.ds(q0, QCHUNK)
                nc.sync.dma_start(
                    out=ag_ins[qc][:].rearrange("(m p) q -> p m q", p=128),
                    in_=yT[:, :, csl])
                if use_collective:
                    nc.gpsimd.collective_compute(
                        "AllGather", mybir.AluOpType.bypass,
                        ins=[ag_ins[qc][:]], outs=[ag_outs[qc][:]],
                        replica_groups=[[0, 1, 2, 3], [4, 5, 6, 7]])

            def proj_part(qc):
                q0 = qc * QCHUNK
                csl = bass.ds(q0, QCHUNK)
                if use_collective:
                    nc.sync.dma_start(
                        out=yF[:, :, csl],
                        in_=ag_outs[qc][:].rearrange("(k p) q -> p k q",
                                                     p=128))
                else:
                    for k2 in range(0, CT, 2):
                        nc.sync.dma_start(
                            out=yF[:, k2:k2+2, csl],
                            in_=ag_ins[qc][:].rearrange("(k p) q -> p k q",
                                                        p=128))
                st = stg.tile([128, QCHUNK // 128, DQ], F32, tag="st")
                for mi, m in enumerate(range(q0 // 128,
                                             (q0 + QCHUNK) // 128)):
                    p = ps.tile([128, DQ], F32, tag="psBV", name="psP", bufs=2)
                    for k in range(CT):
                        nc.tensor.matmul(
                            p[:], yF[:, k, m*128:(m+1)*128], wp[:, k, :],
                            start=(k == 0), stop=(k == CT - 1))
                    nc.vector.tensor_copy(st[:, mi, :], p[:])
                nc.scalar.dma_start(
                    out=out_d[q0:q0+QCHUNK, :].rearrange("(m p) d -> p m d",
                                                         p=128),
                    in_=st[:])

            for _rep in range(reps):
                nc.sync.dma_start(
                    out=xT[:], in_=xT_d[:].rearrange("(k p) t -> p k t",
                                                     p=128))
                v_proj()
                qk_proj(0)
                qk_proj(1)
                if phase == "proj":
                    continue
                # out-projection of chunk qc-1 is emitted between the two
                # attention halves of chunk qc, so the AllGather's DRAM
                # round-trip is covered by attention compute instead of
                # stalling the in-order PE queue.
                for qc in range(QC):
                    attention(0, qc)
                    if phase != "attn" and qc > 0:
                        proj_part(qc - 1)
                    attention(1, qc)
                    if phase != "attn":
                        ag_part(qc)
                if phase != "attn":
                    proj_part(QC - 1)

    nc.compile()
    return nc


def host_inputs(x, attn_mask, Wq, Wk, Wv, Wp):
    x = np.asarray(x)
    attn_mask = np.asarray(attn_mask)
    Wq, Wk, Wv, Wp = (np.asarray(a) for a in (Wq, Wk, Wv, Wp))

    def bfc(a):
        return np.ascontiguousarray(
            np.asarray(a, dtype=np.float32)).astype(ml_dtypes.bfloat16)

    ident = np.eye(128, dtype=np.float32).astype(ml_dtypes.bfloat16)
    r = np.arange(128)
    madd = np.where(r[:, None] > r[None, :], 8.0 * NEG, 0.0).astype(np.float32)
    madd = madd.astype(ml_dtypes.bfloat16)
    in_maps = []
    for c in range(N_CORES):
        b, g = c // 4, c % 4
        sl = slice(DQ * g, DQ * (g + 1))
        km = np.where(attn_mask[b] != 0, 0.0, NEG).astype(np.float32)
        km = np.ascontiguousarray(km.reshape(NB, 128).T)
        in_maps.append({
            "xT": bfc(x[b].T),
            "wqT": bfc(Wq[sl, :].T),
            "wkT": bfc(Wk[sl, :].T),
            "wvT": bfc(Wv[sl, :].T),
            "wpT": bfc(Wp[sl, :].T),
            "kmask": km,
            "ident": np.ascontiguousarray(ident),
            "maskadd": np.ascontiguousarray(madd),
        })
    return in_maps


_CACHED = {}


def kernel(x, attn_mask, Wq, Wk, Wv, Wp):
    if "nc" not in _CACHED:
        _CACHED["nc"] = build_kernel()
    nc = _CACHED["nc"]
    in_maps = host_inputs(x, attn_mask, Wq, Wk, Wv, Wp)
    res = bass_utils.run_bass_kernel_spmd(
        nc, in_maps, core_ids=list(range(N_CORES)))
    out = np.zeros((B, L, C), np.float32)
    for b in range(B):
        out[b] = np.concatenate(
            [res.results[4*b + g]["out"] for g in range(4)], axis=1)
    return out

